# revision 30
# baseline (speedup 1.0000x reference)
"""BartSentenceAttention Trainium2 kernel.

Data-parallel over batch: B=8 batches, one NeuronCore each.

Math restructure (per batch b):
  - Each sentence bucket m holds exactly the hidden state at the position of
    the (m+1)-th MASK token, so `pooled` is a host-side gather (indices come
    from the tiny input_ids tensor), zero-filled past the sentence count.
  - static = pooled @ Wv + bv ; k = static @ Wk + bk ; v = static @ Wvt + bvt
  - logits = (h @ Wq + bq) * s @ k^T  ==  h @ Wqk + blogit,
        Wqk   = s * (Wq @ k^T)            [1024, 128]   (tiny, on device)
        blogit= s * (k @ bq)              [128]
  - out = softmax_masked(logits) @ (v @ Wo) + bo = (masked_exp @ Wvo) * recip
        Wvo = v @ Wo                      [128, 1024]
  - Attention kept in [m, t] (sentence-major) layout on chip: softmax's
    reduction over m becomes a ones-vector matmul on the PE; the 1/sum
    normalization rides the PSUM->SBUF copy as a per-partition ACT scale.

HBM traffic per core per execution: h^T bf16 (16 MiB) in + out bf16
(16 MiB, upcast to f32 on host) + ~2.5 MiB one-time aux.  Measured
steady-state ~100-115us/core with all 8 cores saturating the chip's HBM.
"""

import numpy as np
import ml_dtypes

import concourse.bass as bass
import concourse.tile as tile
from concourse import bacc, mybir
from concourse.bass2jax import _bass_exec_p, install_neuronx_cc_hook

B, S, E, D, M = 8, 8192, 1024, 128, 128
MASK_ID = 3
TT = 512               # tokens per main-loop tile
NTILES = S // TT
NSUB = TT // 128       # 128-token subtiles per tile
NCH = E // 128         # contraction chunks
SCALE = float(D) ** -0.5
F32 = mybir.dt.float32
BF16 = mybir.dt.bfloat16
OUT_BF16 = True          # store output as bf16, upcast to f32 on host
MASK_ON_GPSIMD = True
OUT_DT = BF16 if OUT_BF16 else F32
BF16_NP = ml_dtypes.bfloat16


def build_program(has_bo: bool, repeat: int = 1, mode: str = "full"):
    """mode: 'full' | 'dma' (loads+stores only) | 'noout' (no stores) |
    'noin' (loads replaced by reuse of first tile)."""
    nc = bacc.Bacc("TRN2", target_bir_lowering=False, debug=False,
                   enable_asserts=False)

    hT = nc.dram_tensor("hT", [NTILES, 128, NCH, TT], BF16,
                        kind="ExternalInput")
    maskT = nc.dram_tensor("maskT", [M, S], BF16, kind="ExternalInput")
    pooledT = nc.dram_tensor("pooledT", [E, M], F32, kind="ExternalInput")
    WqT = nc.dram_tensor("WqT", [D, E], F32, kind="ExternalInput")
    Wv = nc.dram_tensor("Wv", [E, D], F32, kind="ExternalInput")
    Wk = nc.dram_tensor("Wk", [D, D], F32, kind="ExternalInput")
    Wvt = nc.dram_tensor("Wvt", [D, D], F32, kind="ExternalInput")
    Wo = nc.dram_tensor("Wo", [D, E], F32, kind="ExternalInput")
    bv = nc.dram_tensor("bv", [D, 1], F32, kind="ExternalInput")
    bk = nc.dram_tensor("bk", [D, 1], F32, kind="ExternalInput")
    bvt = nc.dram_tensor("bvt", [D, 1], F32, kind="ExternalInput")
    bq = nc.dram_tensor("bq", [D, 1], F32, kind="ExternalInput")
    bo = nc.dram_tensor("bo", [1, E], F32, kind="ExternalInput")
    out = nc.dram_tensor("out", [NTILES, 128, NSUB, E], OUT_DT,
                         kind="ExternalOutput")

    AF = mybir.ActivationFunctionType

    with tile.TileContext(nc) as tc:
        with tc.tile_pool(name="persist", bufs=1) as persist:
            wqk_sb = persist.tile([128, NCH, 128], BF16)      # [e%128, chunk, m]
            wvo_sb = persist.tile([128, E], BF16)             # [m, e]
            blogit_sb = persist.tile([128, 1], F32)           # [m, 1]
            ones_sb = persist.tile([128, 1], BF16)
            maskT_sb = persist.tile([128, S], BF16)           # [m, t]
            bo_sb = (persist.tile([128, E], F32, name="bo_sb")
                     if has_bo else None)

            nc.vector.memset(ones_sb, 1.0)
            nc.sync.dma_start(out=maskT_sb, in_=maskT.ap())
            if has_bo:
                bo_bcast = bass.AP(
                    tensor=bo.ap().tensor, offset=0,
                    ap=[[0, 128], [1, E]])
                nc.gpsimd.dma_start(out=bo_sb, in_=bo_bcast)

            # ---------------- per-batch precompute (all tiny, f32) ---------
            with tc.tile_pool(name="pre", bufs=1) as pre, \
                 tc.tile_pool(name="pre_ps", bufs=2, space="PSUM") as pre_ps:
                wv_sb = pre.tile([128, NCH, 128], F32)
                pt_sb = pre.tile([128, NCH, 128], F32)
                wqT_sb = pre.tile([128, E], F32)
                wk_sb = pre.tile([128, 128], F32)
                wvt_sb = pre.tile([128, 128], F32)
                wo_sb = pre.tile([128, E], F32)
                bv_sb = pre.tile([128, 1], F32)
                bk_sb = pre.tile([128, 1], F32)
                bvt_sb = pre.tile([128, 1], F32)
                bq_sb = pre.tile([128, 1], F32)

                nc.sync.dma_start(out=wv_sb,
                                  in_=Wv.ap().rearrange("(c p) d -> p c d", p=128))
                nc.sync.dma_start(out=pt_sb,
                                  in_=pooledT.ap().rearrange("(c p) m -> p c m", p=128))
                nc.sync.dma_start(out=wqT_sb, in_=WqT.ap())
                nc.sync.dma_start(out=wk_sb, in_=Wk.ap())
                nc.sync.dma_start(out=wvt_sb, in_=Wvt.ap())
                nc.sync.dma_start(out=wo_sb, in_=Wo.ap())
                nc.sync.dma_start(out=bv_sb, in_=bv.ap())
                nc.sync.dma_start(out=bk_sb, in_=bk.ap())
                nc.sync.dma_start(out=bvt_sb, in_=bvt.ap())
                nc.sync.dma_start(out=bq_sb, in_=bq.ap())

                # staticT[d, m] = (pooled @ Wv)^T + bv
                ps_st = pre_ps.tile([128, 128], F32, tag="ps128")
                for c in range(NCH):
                    nc.tensor.matmul(ps_st, lhsT=wv_sb[:, c, :], rhs=pt_sb[:, c, :],
                                     start=(c == 0), stop=(c == NCH - 1))
                staticT_sb = pre.tile([128, 128], F32)
                nc.vector.tensor_scalar_add(out=staticT_sb, in0=ps_st,
                                            scalar1=bv_sb)

                # k^T[d', m], v^T[d', m]
                ps_k = pre_ps.tile([128, 128], F32, tag="ps128")
                nc.tensor.matmul(ps_k, lhsT=wk_sb, rhs=staticT_sb,
                                 start=True, stop=True)
                ksenT_sb = pre.tile([128, 128], F32)
                nc.vector.tensor_scalar_add(out=ksenT_sb, in0=ps_k,
                                            scalar1=bk_sb)

                ps_v = pre_ps.tile([128, 128], F32, tag="ps128")
                nc.tensor.matmul(ps_v, lhsT=wvt_sb, rhs=staticT_sb,
                                 start=True, stop=True)
                vsenT_sb = pre.tile([128, 128], F32)
                nc.vector.tensor_scalar_add(out=vsenT_sb, in0=ps_v,
                                            scalar1=bvt_sb)

                # Wqk[e, m] = s * Wq @ k^T   (chunked over e)
                for c in range(NCH):
                    ps_q = pre_ps.tile([128, 128], F32, tag="ps128")
                    nc.tensor.matmul(ps_q, lhsT=wqT_sb[:, c * 128:(c + 1) * 128],
                                     rhs=ksenT_sb, start=True, stop=True)
                    nc.scalar.activation(wqk_sb[:, c, :], ps_q, AF.Copy,
                                         scale=SCALE)

                # Wvo[m, e] = v @ Wo
                for h in range(2):
                    ps_o = pre_ps.tile([128, 512], F32, tag="ps512")
                    nc.tensor.matmul(ps_o, lhsT=vsenT_sb,
                                     rhs=wo_sb[:, h * 512:(h + 1) * 512],
                                     start=True, stop=True)
                    nc.scalar.activation(wvo_sb[:, h * 512:(h + 1) * 512], ps_o,
                                         AF.Copy, scale=1.0)

                # blogit[m] = s * k @ bq
                ps_b = pre_ps.tile([128, 1], F32, tag="ps1")
                nc.tensor.matmul(ps_b, lhsT=ksenT_sb, rhs=bq_sb,
                                 start=True, stop=True)
                nc.vector.tensor_scalar_mul(out=blogit_sb, in0=ps_b,
                                            scalar1=SCALE)

            # ---------------- main loop over token tiles -------------------
            hT_ap = hT.ap()
            out_ap = out.ap()

            with tc.tile_pool(name="loop", bufs=3) as loop, \
                 tc.tile_pool(name="outp", bufs=3) as outp, \
                 tc.tile_pool(name="ps_l", bufs=2, space="PSUM") as ps_l, \
                 tc.tile_pool(name="ps_o", bufs=2, space="PSUM") as ps_o, \
                 tc.tile_pool(name="ps_s", bufs=2, space="PSUM") as ps_s:
                if mode == "dma":
                    o_dummy = persist.tile([128, NSUB, E], OUT_DT)
                    nc.vector.memset(o_dummy, 0.0)
                if mode in ("noin", "compute"):
                    h_fix = persist.tile([128, NCH, TT], BF16)
                    nc.sync.dma_start(out=h_fix, in_=hT_ap[0])
                for it in range(NTILES * repeat):
                    ti = it % NTILES
                    t0 = ti * TT
                    if mode in ("noin", "compute"):
                        h_sb = h_fix
                    else:
                        h_sb = loop.tile([128, NCH, TT], BF16, tag="h")
                        nc.sync.dma_start(out=h_sb, in_=hT_ap[ti])
                    out_r = out_ap[ti]
                    if mode == "dma":
                        nc.scalar.dma_start(out=out_r, in_=o_dummy)
                        continue

                    # logitsT[m, t] = Wqk^T @ hT  (accumulate over e chunks)
                    pl = ps_l.tile([128, TT], F32, tag="logits")
                    for c in range(NCH):
                        nc.tensor.matmul(pl, lhsT=wqk_sb[:, c, :],
                                         rhs=h_sb[:, c, :],
                                         start=(c == 0), stop=(c == NCH - 1))

                    # exp(logits + blogit), then mask
                    exp_sb = loop.tile([128, TT], BF16, tag="exp")
                    nc.scalar.activation(exp_sb, pl, AF.Exp,
                                         bias=blogit_sb, scale=1.0)
                    msk_sb = loop.tile([128, TT], BF16, tag="msk")
                    mask_engine = nc.gpsimd if MASK_ON_GPSIMD else nc.vector
                    mask_engine.tensor_mul(msk_sb, exp_sb,
                                           maskT_sb[:, t0:t0 + TT])

                    # Per subtile: column-sum (ones-matmul) + both output
                    # matmuls share one stationary load of msk[:,sub]; the
                    # 1/sum then rides the PSUM->SBUF copy as an ACT scale.
                    ps_sum = ps_s.tile([128, NSUB], F32, tag="sums")
                    sums_sb = loop.tile([128, NSUB], F32, tag="sumsb")
                    recip_sb = loop.tile([128, NSUB], F32, tag="recip")
                    o_sb = outp.tile([128, NSUB, E], OUT_DT, tag="osb")
                    for sub in range(NSUB):
                        lhs = msk_sb[:, sub * 128:(sub + 1) * 128]
                        nc.tensor.matmul(ps_sum[:, sub:sub + 1], lhsT=lhs,
                                         rhs=ones_sb, start=True, stop=True)
                        po = ps_o.tile([128, E], F32, tag="out")
                        nc.tensor.matmul(po[:, 0:512], lhsT=lhs,
                                         rhs=wvo_sb[:, 0:512],
                                         start=True, stop=True)
                        nc.tensor.matmul(po[:, 512:1024], lhsT=lhs,
                                         rhs=wvo_sb[:, 512:1024],
                                         start=True, stop=True)
                        nc.vector.tensor_scalar_max(
                            out=sums_sb[:, sub:sub + 1],
                            in0=ps_sum[:, sub:sub + 1], scalar1=1e-30)
                        nc.vector.reciprocal(out=recip_sb[:, sub:sub + 1],
                                             in_=sums_sb[:, sub:sub + 1])
                        r = recip_sb[:, sub:sub + 1]
                        if sub % 2 == 0:
                            nc.scalar.activation(o_sb[:, sub, :], po,
                                                 AF.Copy, scale=r)
                        else:
                            nc.vector.tensor_scalar_mul(out=o_sb[:, sub, :],
                                                        in0=po, scalar1=r)
                        if has_bo:
                            nc.vector.tensor_add(o_sb[:, sub, :],
                                                 o_sb[:, sub, :], bo_sb)
                    if mode not in ("noout", "compute"):
                        nc.scalar.dma_start(out=out_r, in_=o_sb)

    nc.compile()
    return nc


def prepare_inputs(input_ids, hidden_states, Wv, bv, Wq, bq, Wk, bk,
                   Wvt, bvt, Wo, bo):
    """Host-side prep: mask indices from input_ids, transposes, bf16 casts.

    Returns (in_maps, has_bo, csum) where in_maps[core] feeds core `core`.
    """
    ids = np.asarray(input_ids)
    h = np.asarray(hidden_states, dtype=np.float32)
    Wv = np.asarray(Wv, np.float32)
    Wq = np.asarray(Wq, np.float32)
    Wk = np.asarray(Wk, np.float32)
    Wvt = np.asarray(Wvt, np.float32)
    Wo = np.asarray(Wo, np.float32)
    bv = np.asarray(bv, np.float32).reshape(D, 1)
    bq_ = np.asarray(bq, np.float32).reshape(D, 1)
    bk_ = np.asarray(bk, np.float32).reshape(D, 1)
    bvt_ = np.asarray(bvt, np.float32).reshape(D, 1)
    bo_ = np.asarray(bo, np.float32).reshape(1, E)
    has_bo = bool(np.any(bo_ != 0))

    mask = ids == MASK_ID
    csum = np.cumsum(mask, axis=1).astype(np.int32)          # (B, S)
    WqT = np.ascontiguousarray(Wq.T)                          # (D, E)

    shared = dict(WqT=WqT, Wv=Wv, Wk=Wk, Wvt=Wvt, Wo=Wo,
                  bv=bv, bk=bk_, bvt=bvt_, bq=bq_, bo=bo_)

    m_plus_1 = np.arange(1, M + 1, dtype=np.int32)[:, None]   # (M, 1)
    in_maps = []
    for b in range(B):
        pos = np.flatnonzero(mask[b])[:M]
        pooled = np.zeros((M, E), np.float32)
        pooled[:len(pos)] = h[b, pos]
        pooledT = np.ascontiguousarray(pooled.T)              # (E, M)
        hTb = h[b].T.astype(BF16_NP)                          # (E, S)
        hT = np.ascontiguousarray(
            hTb.reshape(NCH, 128, NTILES, TT).transpose(2, 1, 0, 3))
        maskT = (m_plus_1 <= csum[b][None, :]).astype(BF16_NP)  # (M, S)
        in_maps.append(dict(hT=hT, maskT=maskT, pooledT=pooledT, **shared))
    return in_maps, has_bo, csum


class Runner:
    """Compiled SPMD executable over jax.devices()[:B] (adapted from
    concourse.bass2jax.run_bass_via_pjrt, kept callable for re-runs)."""

    def __init__(self, nc):
        import jax
        from jax.sharding import Mesh, PartitionSpec
        from jax.experimental.shard_map import shard_map
        from concourse.bass2jax import partition_id_tensor

        install_neuronx_cc_hook()
        self.nc = nc
        partition_name = (nc.partition_id_tensor.name
                          if nc.partition_id_tensor else None)
        in_names, out_names, out_avals, zero_outs = [], [], [], []
        for alloc in nc.m.functions[0].allocations:
            if not isinstance(alloc, mybir.MemoryLocationSet):
                continue
            name = alloc.memorylocations[0].name
            if alloc.kind == "ExternalInput":
                if name != partition_name:
                    in_names.append(name)
            elif alloc.kind == "ExternalOutput":
                out_names.append(name)
                shape = tuple(alloc.tensor_shape)
                dtype = mybir.dt.np(alloc.dtype)
                out_avals.append(jax.core.ShapedArray(shape, dtype))
                zero_outs.append(np.zeros(shape, dtype))
        self.in_names, self.out_names = in_names, out_names
        n_params, n_outs = len(in_names), len(out_names)

        extra_names = [partition_name] if partition_name else []

        def _body(*args):
            operands = list(args)
            if partition_name:
                operands.append(partition_id_tensor())
            outs = _bass_exec_p.bind(
                *operands,
                out_avals=tuple(out_avals),
                in_names=tuple(in_names + out_names + extra_names),
                out_names=tuple(out_names),
                lowering_input_output_aliases=(),
                sim_require_finite=True,
                sim_require_nnan=True,
                nc=nc,
            )
            return tuple(outs)

        devices = jax.devices()[:B]
        assert len(devices) == B, f"need {B} cores, have {len(jax.devices())}"
        mesh = Mesh(np.asarray(devices), ("core",))
        self.mesh = mesh
        self.jax = jax
        self.out_avals = out_avals
        self.zero_outs = zero_outs
        self.n_params, self.n_outs = n_params, n_outs
        pspec = PartitionSpec("core")
        self.pspec = pspec
        self._shard_map, self._partition_name = shard_map, partition_name
        self._partition_id_tensor = partition_id_tensor
        self.sharded = jax.jit(
            shard_map(_body, mesh=mesh,
                      in_specs=(pspec,) * (n_params + n_outs),
                      out_specs=(pspec,) * n_outs,
                      check_rep=False),
            keep_unused=True,
        )


    def put_inputs(self, in_maps):
        """Concat per-core inputs on axis 0 and move to devices once."""
        import jax
        from jax.sharding import NamedSharding
        sharding = NamedSharding(self.mesh, self.pspec)
        args = []
        for name in self.in_names:
            cat = np.concatenate([np.asarray(m[name]) for m in in_maps], axis=0)
            args.append(jax.device_put(cat, sharding))
        for z in self.zero_outs:
            cat = np.zeros((B * z.shape[0], *z.shape[1:]), z.dtype)
            args.append(jax.device_put(cat, sharding))
        return args

    def __call__(self, args):
        return self.sharded(*args)

    def run(self, in_maps):
        out_arrs = self(self.put_inputs(in_maps))
        res = []
        for c in range(B):
            res.append({
                name: np.asarray(out_arrs[i]).reshape(
                    B, *self.out_avals[i].shape)[c]
                for i, name in enumerate(self.out_names)
            })
        return res


_CACHE = {}


def get_runner(has_bo: bool, repeat: int = 1, mode: str = "full") -> Runner:
    key = (has_bo, repeat, mode)
    if key not in _CACHE:
        _CACHE[key] = Runner(build_program(has_bo, repeat, mode))
    return _CACHE[key]


def kernel(**inputs) -> np.ndarray:
    in_maps, has_bo, _ = prepare_inputs(**inputs)
    runner = get_runner(has_bo)
    res = runner.run(in_maps)
    outs = []
    for b in range(B):
        o = res[b]["out"]                     # [NTILES, 128, NSUB, E]
        o = o.transpose(0, 2, 1, 3).reshape(S, E)
        outs.append(o)
    return np.stack(outs, axis=0).astype(np.float32)



# revision 31
# speedup vs baseline: 1.3317x; 1.3317x over previous
"""BartSentenceAttention Trainium2 kernel.

Data-parallel over batch: B=8 batches, one NeuronCore each.

Math restructure (per batch b):
  - Each sentence bucket m holds exactly the hidden state at the position of
    the (m+1)-th MASK token, so `pooled` is a host-side gather (indices come
    from the tiny input_ids tensor), zero-filled past the sentence count.
  - static = pooled @ Wv + bv ; k = static @ Wk + bk ; v = static @ Wvt + bvt
  - logits = (h @ Wq + bq) * s @ k^T  ==  h @ Wqk + blogit,
        Wqk   = s * (Wq @ k^T)            [1024, 128]   (tiny, on device)
        blogit= s * (k @ bq)              [128]
  - out = softmax_masked(logits) @ (v @ Wo) + bo = (masked_exp @ Wvo) * recip
        Wvo = v @ Wo                      [128, 1024]
  - Attention kept in [m, t] (sentence-major) layout on chip: softmax's
    reduction over m becomes a ones-vector matmul on the PE; the 1/sum
    normalization rides the PSUM->SBUF copy as a per-partition ACT scale.

HBM traffic per core per execution: h^T bf16 (16 MiB) in + out bf16
(16 MiB, upcast to f32 on host) + ~2.5 MiB one-time aux.  Measured
steady-state ~100-115us/core with all 8 cores saturating the chip's HBM.
"""

import numpy as np
import ml_dtypes

import concourse.bass as bass
import concourse.tile as tile
from concourse import bacc, mybir
from concourse.bass2jax import _bass_exec_p, install_neuronx_cc_hook

B, S, E, D, M = 8, 8192, 1024, 128, 128
MASK_ID = 3
TT = 512               # tokens per main-loop tile
NTILES = S // TT
NSUB = TT // 128       # 128-token subtiles per tile
NCH = E // 128         # contraction chunks
SCALE = float(D) ** -0.5
F32 = mybir.dt.float32
BF16 = mybir.dt.bfloat16
OUT_BF16 = True          # store output as bf16, upcast to f32 on host
MASK_ON_GPSIMD = True
TPD = 2                  # 512-token tiles per load/store DMA
OUT_DT = BF16 if OUT_BF16 else F32
BF16_NP = ml_dtypes.bfloat16


def build_program(has_bo: bool, repeat: int = 1, mode: str = "full"):
    """mode: 'full' | 'dma' (loads+stores only) | 'noout' (no stores) |
    'noin' (loads replaced by reuse of first tile)."""
    nc = bacc.Bacc("TRN2", target_bir_lowering=False, debug=False,
                   enable_asserts=False)

    hT = nc.dram_tensor("hT", [NTILES, 128, NCH, TT], BF16,
                        kind="ExternalInput")
    maskT = nc.dram_tensor("maskT", [M, S], BF16, kind="ExternalInput")
    pooledT = nc.dram_tensor("pooledT", [E, M], F32, kind="ExternalInput")
    WqT = nc.dram_tensor("WqT", [D, E], F32, kind="ExternalInput")
    Wv = nc.dram_tensor("Wv", [E, D], F32, kind="ExternalInput")
    Wk = nc.dram_tensor("Wk", [D, D], F32, kind="ExternalInput")
    Wvt = nc.dram_tensor("Wvt", [D, D], F32, kind="ExternalInput")
    Wo = nc.dram_tensor("Wo", [D, E], F32, kind="ExternalInput")
    bv = nc.dram_tensor("bv", [D, 1], F32, kind="ExternalInput")
    bk = nc.dram_tensor("bk", [D, 1], F32, kind="ExternalInput")
    bvt = nc.dram_tensor("bvt", [D, 1], F32, kind="ExternalInput")
    bq = nc.dram_tensor("bq", [D, 1], F32, kind="ExternalInput")
    bo = nc.dram_tensor("bo", [1, E], F32, kind="ExternalInput")
    out = nc.dram_tensor("out", [NTILES, 128, NSUB, E], OUT_DT,
                         kind="ExternalOutput")

    AF = mybir.ActivationFunctionType

    with tile.TileContext(nc) as tc:
        with tc.tile_pool(name="persist", bufs=1) as persist:
            wqk_sb = persist.tile([128, NCH, 128], BF16)      # [e%128, chunk, m]
            wvo_sb = persist.tile([128, E], BF16)             # [m, e]
            blogit_sb = persist.tile([128, 1], F32)           # [m, 1]
            ones_sb = persist.tile([128, 1], BF16)
            maskT_sb = persist.tile([128, S], BF16)           # [m, t]
            bo_sb = (persist.tile([128, E], F32, name="bo_sb")
                     if has_bo else None)

            nc.vector.memset(ones_sb, 1.0)
            nc.sync.dma_start(out=maskT_sb, in_=maskT.ap())
            if has_bo:
                bo_bcast = bass.AP(
                    tensor=bo.ap().tensor, offset=0,
                    ap=[[0, 128], [1, E]])
                nc.gpsimd.dma_start(out=bo_sb, in_=bo_bcast)

            # ---------------- per-batch precompute (all tiny, f32) ---------
            with tc.tile_pool(name="pre", bufs=1) as pre, \
                 tc.tile_pool(name="pre_ps", bufs=2, space="PSUM") as pre_ps:
                wv_sb = pre.tile([128, NCH, 128], F32)
                pt_sb = pre.tile([128, NCH, 128], F32)
                wqT_sb = pre.tile([128, E], F32)
                wk_sb = pre.tile([128, 128], F32)
                wvt_sb = pre.tile([128, 128], F32)
                wo_sb = pre.tile([128, E], F32)
                bv_sb = pre.tile([128, 1], F32)
                bk_sb = pre.tile([128, 1], F32)
                bvt_sb = pre.tile([128, 1], F32)
                bq_sb = pre.tile([128, 1], F32)

                nc.sync.dma_start(out=wv_sb,
                                  in_=Wv.ap().rearrange("(c p) d -> p c d", p=128))
                nc.sync.dma_start(out=pt_sb,
                                  in_=pooledT.ap().rearrange("(c p) m -> p c m", p=128))
                nc.sync.dma_start(out=wqT_sb, in_=WqT.ap())
                nc.sync.dma_start(out=wk_sb, in_=Wk.ap())
                nc.sync.dma_start(out=wvt_sb, in_=Wvt.ap())
                nc.sync.dma_start(out=wo_sb, in_=Wo.ap())
                nc.sync.dma_start(out=bv_sb, in_=bv.ap())
                nc.sync.dma_start(out=bk_sb, in_=bk.ap())
                nc.sync.dma_start(out=bvt_sb, in_=bvt.ap())
                nc.sync.dma_start(out=bq_sb, in_=bq.ap())

                # staticT[d, m] = (pooled @ Wv)^T + bv
                ps_st = pre_ps.tile([128, 128], F32, tag="ps128")
                for c in range(NCH):
                    nc.tensor.matmul(ps_st, lhsT=wv_sb[:, c, :], rhs=pt_sb[:, c, :],
                                     start=(c == 0), stop=(c == NCH - 1))
                staticT_sb = pre.tile([128, 128], F32)
                nc.vector.tensor_scalar_add(out=staticT_sb, in0=ps_st,
                                            scalar1=bv_sb)

                # k^T[d', m], v^T[d', m]
                ps_k = pre_ps.tile([128, 128], F32, tag="ps128")
                nc.tensor.matmul(ps_k, lhsT=wk_sb, rhs=staticT_sb,
                                 start=True, stop=True)
                ksenT_sb = pre.tile([128, 128], F32)
                nc.vector.tensor_scalar_add(out=ksenT_sb, in0=ps_k,
                                            scalar1=bk_sb)

                ps_v = pre_ps.tile([128, 128], F32, tag="ps128")
                nc.tensor.matmul(ps_v, lhsT=wvt_sb, rhs=staticT_sb,
                                 start=True, stop=True)
                vsenT_sb = pre.tile([128, 128], F32)
                nc.vector.tensor_scalar_add(out=vsenT_sb, in0=ps_v,
                                            scalar1=bvt_sb)

                # Wqk[e, m] = s * Wq @ k^T   (chunked over e)
                for c in range(NCH):
                    ps_q = pre_ps.tile([128, 128], F32, tag="ps128")
                    nc.tensor.matmul(ps_q, lhsT=wqT_sb[:, c * 128:(c + 1) * 128],
                                     rhs=ksenT_sb, start=True, stop=True)
                    nc.scalar.activation(wqk_sb[:, c, :], ps_q, AF.Copy,
                                         scale=SCALE)

                # Wvo[m, e] = v @ Wo
                for h in range(2):
                    ps_o = pre_ps.tile([128, 512], F32, tag="ps512")
                    nc.tensor.matmul(ps_o, lhsT=vsenT_sb,
                                     rhs=wo_sb[:, h * 512:(h + 1) * 512],
                                     start=True, stop=True)
                    nc.scalar.activation(wvo_sb[:, h * 512:(h + 1) * 512], ps_o,
                                         AF.Copy, scale=1.0)

                # blogit[m] = s * k @ bq
                ps_b = pre_ps.tile([128, 1], F32, tag="ps1")
                nc.tensor.matmul(ps_b, lhsT=ksenT_sb, rhs=bq_sb,
                                 start=True, stop=True)
                nc.vector.tensor_scalar_mul(out=blogit_sb, in0=ps_b,
                                            scalar1=SCALE)

            # ---------------- main loop over token tiles -------------------
            # TPD consecutive 512-token tiles share one load DMA and one
            # store DMA (the pre-tiled HBM layout keeps them contiguous).
            hT_ap = hT.ap()
            out_ap = out.ap()
            NSUP = NTILES // TPD

            with tc.tile_pool(name="loop", bufs=3) as loop, \
                 tc.tile_pool(name="outp", bufs=3) as outp, \
                 tc.tile_pool(name="ps_l", bufs=2, space="PSUM") as ps_l, \
                 tc.tile_pool(name="ps_o", bufs=2, space="PSUM") as ps_o, \
                 tc.tile_pool(name="ps_s", bufs=2, space="PSUM") as ps_s:
                if mode == "dma":
                    o_dummy = persist.tile([128, TPD, NSUB, E], OUT_DT)
                    nc.vector.memset(o_dummy, 0.0)
                if mode in ("noin", "compute"):
                    h_fix = persist.tile([128, TPD, NCH, TT], BF16)
                    nc.sync.dma_start(
                        out=h_fix,
                        in_=hT_ap[0:TPD].rearrange("n p c t -> p n c t"))
                for it in range(NSUP * repeat):
                    si = it % NSUP
                    out_r = out_ap[si * TPD:(si + 1) * TPD].rearrange(
                        "n p s e -> p n s e")
                    if mode in ("noin", "compute"):
                        h_sb = h_fix
                    else:
                        h_sb = loop.tile([128, TPD, NCH, TT], BF16, tag="h")
                        nc.sync.dma_start(
                            out=h_sb,
                            in_=hT_ap[si * TPD:(si + 1) * TPD].rearrange(
                                "n p c t -> p n c t"))
                    if mode == "dma":
                        nc.scalar.dma_start(out=out_r, in_=o_dummy)
                        continue

                    o_sb = outp.tile([128, TPD, NSUB, E], OUT_DT, tag="osb")
                    for j in range(TPD):
                        ti = si * TPD + j
                        t0 = ti * TT

                        # logitsT[m, t] = Wqk^T @ hT  (accumulate e chunks)
                        pl = ps_l.tile([128, TT], F32, tag="logits")
                        for c in range(NCH):
                            nc.tensor.matmul(pl, lhsT=wqk_sb[:, c, :],
                                             rhs=h_sb[:, j, c, :],
                                             start=(c == 0),
                                             stop=(c == NCH - 1))

                        # exp(logits + blogit), then mask
                        exp_sb = loop.tile([128, TT], BF16, tag="exp")
                        nc.scalar.activation(exp_sb, pl, AF.Exp,
                                             bias=blogit_sb, scale=1.0)
                        msk_sb = loop.tile([128, TT], BF16, tag="msk")
                        mask_engine = nc.gpsimd if MASK_ON_GPSIMD else nc.vector
                        mask_engine.tensor_mul(msk_sb, exp_sb,
                                               maskT_sb[:, t0:t0 + TT])

                        # Per subtile: column-sum (ones-matmul) + both output
                        # matmuls share one stationary load of msk[:,sub]; the
                        # 1/sum rides the PSUM->SBUF copy as an ACT/DVE scale.
                        # max+recip batched per subtile pair to cut DVE ops.
                        ps_sum = ps_s.tile([128, NSUB], F32, tag="sums")
                        sums_sb = loop.tile([128, NSUB], F32, tag="sumsb")
                        recip_sb = loop.tile([128, NSUB], F32, tag="recip")
                        pos = {}
                        for sub in range(NSUB):
                            lhs = msk_sb[:, sub * 128:(sub + 1) * 128]
                            nc.tensor.matmul(ps_sum[:, sub:sub + 1], lhsT=lhs,
                                             rhs=ones_sb, start=True, stop=True)
                            po = ps_o.tile([128, E], F32, tag="out")
                            nc.tensor.matmul(po[:, 0:512], lhsT=lhs,
                                             rhs=wvo_sb[:, 0:512],
                                             start=True, stop=True)
                            nc.tensor.matmul(po[:, 512:1024], lhsT=lhs,
                                             rhs=wvo_sb[:, 512:1024],
                                             start=True, stop=True)
                            pos[sub] = po
                            if sub % 2 == 0:
                                continue
                            pair = slice(sub - 1, sub + 1)
                            nc.vector.tensor_scalar_max(
                                out=sums_sb[:, pair],
                                in0=ps_sum[:, pair], scalar1=1e-30)
                            nc.vector.reciprocal(out=recip_sb[:, pair],
                                                 in_=sums_sb[:, pair])
                            for s2 in (sub - 1, sub):
                                po2 = pos.pop(s2)
                                r = recip_sb[:, s2:s2 + 1]
                                if s2 % 2 == 0:
                                    nc.scalar.activation(o_sb[:, j, s2, :], po2,
                                                         AF.Copy, scale=r)
                                else:
                                    nc.vector.tensor_scalar_mul(
                                        out=o_sb[:, j, s2, :],
                                        in0=po2, scalar1=r)
                                if has_bo:
                                    nc.vector.tensor_add(o_sb[:, j, s2, :],
                                                         o_sb[:, j, s2, :],
                                                         bo_sb)
                    if mode not in ("noout", "compute"):
                        nc.scalar.dma_start(out=out_r, in_=o_sb)

    nc.compile()
    return nc


def prepare_inputs(input_ids, hidden_states, Wv, bv, Wq, bq, Wk, bk,
                   Wvt, bvt, Wo, bo):
    """Host-side prep: mask indices from input_ids, transposes, bf16 casts.

    Returns (in_maps, has_bo, csum) where in_maps[core] feeds core `core`.
    """
    ids = np.asarray(input_ids)
    h = np.asarray(hidden_states, dtype=np.float32)
    Wv = np.asarray(Wv, np.float32)
    Wq = np.asarray(Wq, np.float32)
    Wk = np.asarray(Wk, np.float32)
    Wvt = np.asarray(Wvt, np.float32)
    Wo = np.asarray(Wo, np.float32)
    bv = np.asarray(bv, np.float32).reshape(D, 1)
    bq_ = np.asarray(bq, np.float32).reshape(D, 1)
    bk_ = np.asarray(bk, np.float32).reshape(D, 1)
    bvt_ = np.asarray(bvt, np.float32).reshape(D, 1)
    bo_ = np.asarray(bo, np.float32).reshape(1, E)
    has_bo = bool(np.any(bo_ != 0))

    mask = ids == MASK_ID
    csum = np.cumsum(mask, axis=1).astype(np.int32)          # (B, S)
    WqT = np.ascontiguousarray(Wq.T)                          # (D, E)

    shared = dict(WqT=WqT, Wv=Wv, Wk=Wk, Wvt=Wvt, Wo=Wo,
                  bv=bv, bk=bk_, bvt=bvt_, bq=bq_, bo=bo_)

    m_plus_1 = np.arange(1, M + 1, dtype=np.int32)[:, None]   # (M, 1)
    in_maps = []
    for b in range(B):
        pos = np.flatnonzero(mask[b])[:M]
        pooled = np.zeros((M, E), np.float32)
        pooled[:len(pos)] = h[b, pos]
        pooledT = np.ascontiguousarray(pooled.T)              # (E, M)
        hTb = h[b].T.astype(BF16_NP)                          # (E, S)
        hT = np.ascontiguousarray(
            hTb.reshape(NCH, 128, NTILES, TT).transpose(2, 1, 0, 3))
        maskT = (m_plus_1 <= csum[b][None, :]).astype(BF16_NP)  # (M, S)
        in_maps.append(dict(hT=hT, maskT=maskT, pooledT=pooledT, **shared))
    return in_maps, has_bo, csum


class Runner:
    """Compiled SPMD executable over jax.devices()[:B] (adapted from
    concourse.bass2jax.run_bass_via_pjrt, kept callable for re-runs)."""

    def __init__(self, nc):
        import jax
        from jax.sharding import Mesh, PartitionSpec
        from jax.experimental.shard_map import shard_map
        from concourse.bass2jax import partition_id_tensor

        install_neuronx_cc_hook()
        self.nc = nc
        partition_name = (nc.partition_id_tensor.name
                          if nc.partition_id_tensor else None)
        in_names, out_names, out_avals, zero_outs = [], [], [], []
        for alloc in nc.m.functions[0].allocations:
            if not isinstance(alloc, mybir.MemoryLocationSet):
                continue
            name = alloc.memorylocations[0].name
            if alloc.kind == "ExternalInput":
                if name != partition_name:
                    in_names.append(name)
            elif alloc.kind == "ExternalOutput":
                out_names.append(name)
                shape = tuple(alloc.tensor_shape)
                dtype = mybir.dt.np(alloc.dtype)
                out_avals.append(jax.core.ShapedArray(shape, dtype))
                zero_outs.append(np.zeros(shape, dtype))
        self.in_names, self.out_names = in_names, out_names
        n_params, n_outs = len(in_names), len(out_names)

        extra_names = [partition_name] if partition_name else []

        def _body(*args):
            operands = list(args)
            if partition_name:
                operands.append(partition_id_tensor())
            outs = _bass_exec_p.bind(
                *operands,
                out_avals=tuple(out_avals),
                in_names=tuple(in_names + out_names + extra_names),
                out_names=tuple(out_names),
                lowering_input_output_aliases=(),
                sim_require_finite=True,
                sim_require_nnan=True,
                nc=nc,
            )
            return tuple(outs)

        devices = jax.devices()[:B]
        assert len(devices) == B, f"need {B} cores, have {len(jax.devices())}"
        mesh = Mesh(np.asarray(devices), ("core",))
        self.mesh = mesh
        self.jax = jax
        self.out_avals = out_avals
        self.zero_outs = zero_outs
        self.n_params, self.n_outs = n_params, n_outs
        pspec = PartitionSpec("core")
        self.pspec = pspec
        self._shard_map, self._partition_name = shard_map, partition_name
        self._partition_id_tensor = partition_id_tensor
        self.sharded = jax.jit(
            shard_map(_body, mesh=mesh,
                      in_specs=(pspec,) * (n_params + n_outs),
                      out_specs=(pspec,) * n_outs,
                      check_rep=False),
            keep_unused=True,
        )


    def put_inputs(self, in_maps):
        """Concat per-core inputs on axis 0 and move to devices once."""
        import jax
        from jax.sharding import NamedSharding
        sharding = NamedSharding(self.mesh, self.pspec)
        args = []
        for name in self.in_names:
            cat = np.concatenate([np.asarray(m[name]) for m in in_maps], axis=0)
            args.append(jax.device_put(cat, sharding))
        for z in self.zero_outs:
            cat = np.zeros((B * z.shape[0], *z.shape[1:]), z.dtype)
            args.append(jax.device_put(cat, sharding))
        return args

    def __call__(self, args):
        return self.sharded(*args)

    def run(self, in_maps):
        out_arrs = self(self.put_inputs(in_maps))
        res = []
        for c in range(B):
            res.append({
                name: np.asarray(out_arrs[i]).reshape(
                    B, *self.out_avals[i].shape)[c]
                for i, name in enumerate(self.out_names)
            })
        return res


_CACHE = {}


def get_runner(has_bo: bool, repeat: int = 1, mode: str = "full") -> Runner:
    key = (has_bo, repeat, mode)
    if key not in _CACHE:
        _CACHE[key] = Runner(build_program(has_bo, repeat, mode))
    return _CACHE[key]


def kernel(**inputs) -> np.ndarray:
    in_maps, has_bo, _ = prepare_inputs(**inputs)
    runner = get_runner(has_bo)
    res = runner.run(in_maps)
    outs = []
    for b in range(B):
        o = res[b]["out"]                     # [NTILES, 128, NSUB, E]
        o = o.transpose(0, 2, 1, 3).reshape(S, E)
        outs.append(o)
    return np.stack(outs, axis=0).astype(np.float32)

